# revision 1
# baseline (speedup 1.0000x reference)
"""Multi-head causal self-attention (B=4, T=1024, d_model=2048, 16 heads of 128)
for 8 Trainium2 NeuronCores.

Sharding: hybrid data x tensor parallel. Core c handles batch b = c//2 and
head group g = c%2 (8 heads per core). Each core computes q/k/v projections
for its 8 heads, causal flash-style attention, and the out-projection rows
for those heads, producing a partial [1024, 2048] output for its batch.
The host sums the two partials per batch and adds the output bias.

All on-device layouts are feature-major so no transposes are needed anywhere:
  - x is shipped pre-transposed per batch: xt [2048, 1024] (fp16)
  - q, k are produced feature-major [dh, T] per head; v token-major [T, dh]
  - scores are computed transposed: S^T[kv, q] = k_fm.T @ q_fm (lhsT=k, rhs=q)
  - softmax denominator via ones[128,128] matmul (partition reduction on PE),
    which also broadcasts the per-q sum to all 128 partitions
  - attention output accumulates as out^T[dh, q] = v_tm.T @ exp(S^T)
  - out^T is exactly the lhsT the out-projection needs

Heads are processed in two blocks of 4 so projection weights and q/k/v
activations fit in SBUF alongside the resident x^T and w_out. Within a
block, attention is computed for two heads interleaved so PE matmuls hide
the ACT exp latency. Inputs are DMA'd in per-k-chunk tiles so the first
projection matmuls start ~2us in instead of waiting for monolithic loads.
"""

import numpy as np

B, T, C = 4, 1024, 2048
H = 16          # total heads
HL = 8          # heads per core (local)
HB = 4          # heads per block
DH = 128        # head dim
KC = C // 128   # contraction chunks (16)
P = 128
NCORES = 8

_cache = {}


def _build():
    import concourse.bacc as bacc
    import concourse.mybir as mybir
    import concourse.tile as tile

    F32 = mybir.dt.float32
    F16 = mybir.dt.float16
    AF = mybir.ActivationFunctionType
    ALU = mybir.AluOpType

    nc = bacc.Bacc("TRN2", target_bir_lowering=False, debug=False)

    xt_d = nc.dram_tensor("xt", (C, T), F16, kind="ExternalInput")
    wq_d = nc.dram_tensor("wq", (C, HL * DH), F16, kind="ExternalInput")
    wk_d = nc.dram_tensor("wk", (C, HL * DH), F16, kind="ExternalInput")
    wv_d = nc.dram_tensor("wv", (C, HL * DH), F16, kind="ExternalInput")
    wo_d = nc.dram_tensor("wo", (HL * DH, C), F16, kind="ExternalInput")
    bq_d = nc.dram_tensor("bq", (P, HL), F32, kind="ExternalInput")
    bk_d = nc.dram_tensor("bk", (P, HL), F32, kind="ExternalInput")
    bvb_d = nc.dram_tensor("bvb", (P, HL * DH), F32, kind="ExternalInput")
    mask_d = nc.dram_tensor("mask", (P, P), F32, kind="ExternalInput")
    part_d = nc.dram_tensor("part", (T, C), F32, kind="ExternalOutput")

    BW = HB * DH  # head-block feature width (512)

    xt_v = xt_d.rearrange("(o p) t -> p o t", p=P)
    wq_v = wq_d.rearrange("(o p) m -> p o m", p=P)
    wk_v = wk_d.rearrange("(o p) m -> p o m", p=P)
    wv_v = wv_d.rearrange("(o p) m -> p o m", p=P)

    with tile.TileContext(nc) as tc:
        with (
            tc.tile_pool(name="res", bufs=1) as res,
            tc.tile_pool(name="wblk", bufs=1) as wblk,
            tc.tile_pool(name="qkv", bufs=2) as qkv,
            tc.tile_pool(name="wp", bufs=3) as wp,
            tc.tile_pool(name="ps", bufs=3, space="PSUM") as ps,
        ):
            bq_sb = res.tile([P, HL], F32, tag="bq")
            bk_sb = res.tile([P, HL], F32, tag="bk")
            bvb_sb = res.tile([P, HL * DH], F32, tag="bvb")
            mask_sb = res.tile([P, P], F32, tag="mask")

            ones_sb = res.tile([P, P], F16, tag="ones")
            nc.vector.memset(ones_sb[:], 1.0)

            # Warm the PE (HAM un-throttles after ~3.4us of activity) while the
            # input DMAs stream in; these matmuls depend only on the memset.
            warm = ps.tile([P, P], F32, tag="mm")
            for _ in range(48):
                nc.tensor.matmul(warm[:], ones_sb[:], ones_sb[:], start=True, stop=True)

            # x^T in per-k-chunk tiles so compute starts after the first chunks
            xts = []
            for kc in range(KC):
                xt_sb = res.tile([P, T], F16, tag=f"xt{kc}", name=f"xt{kc}")
                xts.append(xt_sb)
            wts = {w: [None] * KC for w in ("wq", "wk", "wv")}

            def dma_block_weights(blk):
                lo = blk * BW

                def load_w(wname, wv_, kc):
                    wt = wblk.tile(
                        [P, BW], F16, tag=f"{wname}{kc}", name=f"{wname}{kc}_{blk}"
                    )
                    nc.sync.dma_start(wt[:], wv_[:, kc, lo : lo + BW])
                    wts[wname][kc] = wt

                if blk == 0:
                    # arrival order matches first consumption: the h=0 q-proj
                    # k-chain needs (xt[kc], wq[kc]) pairs in kc order
                    for kc in range(KC):
                        nc.sync.dma_start(xts[kc][:], xt_v[:, kc, :])
                        load_w("wq", wq_v, kc)
                    nc.sync.dma_start(bq_sb[:], bq_d[:])
                    nc.sync.dma_start(bk_sb[:], bk_d[:])
                    nc.sync.dma_start(bvb_sb[:], bvb_d[:])
                    nc.sync.dma_start(mask_sb[:], mask_d[:])
                    for kc in range(KC):
                        load_w("wk", wk_v, kc)
                    for kc in range(KC):
                        load_w("wv", wv_v, kc)
                else:
                    for kc in range(KC):
                        load_w("wq", wq_v, kc)
                        load_w("wk", wk_v, kc)
                        load_w("wv", wv_v, kc)

            wo_sb = res.tile([P, HL, C], F16, tag="wo")
            oT = res.tile([P, HL, T], F16, tag="oT")

            for blk in range(HL // HB):
                lo = blk * BW
                dma_block_weights(blk)

                qf = qkv.tile([P, HB, T], F16, tag="qf")
                kf = qkv.tile([P, HB, T], F16, tag="kf")
                vt = qkv.tile([P, T // P, BW], F16, tag="vt")

                # ---- Phase 1: projections for this block ----
                for h in range(HB):
                    for dst, wname, bsb in (("qf", "wq", bq_sb), ("kf", "wk", bk_sb)):
                        dtile = qf if dst == "qf" else kf
                        for t in range(T // 512):
                            pt = ps.tile([P, 512], F32, tag="mm")
                            for kc in range(KC):
                                nc.tensor.matmul(
                                    pt[:],
                                    wts[wname][kc][:, h * DH : (h + 1) * DH],
                                    xts[kc][:, t * 512 : (t + 1) * 512],
                                    start=(kc == 0),
                                    stop=(kc == KC - 1),
                                )
                            nc.vector.tensor_tensor(
                                dtile[:, h, t * 512 : (t + 1) * 512],
                                pt[:],
                                bsb[
                                    :, blk * HB + h : blk * HB + h + 1
                                ].to_broadcast((P, 512)),
                                ALU.add,
                            )
                for m in range(T // P):
                    pt = ps.tile([P, 512], F32, tag="mm")
                    for kc in range(KC):
                        nc.tensor.matmul(
                            pt[:],
                            xts[kc][:, m * P : (m + 1) * P],
                            wts["wv"][kc][:],
                            start=(kc == 0),
                            stop=(kc == KC - 1),
                        )
                    nc.vector.tensor_tensor(
                        vt[:, m, :], pt[:], bvb_sb[:, lo : lo + BW], ALU.add
                    )

                if blk == 0:
                    # out-proj weights: needed only in phase 3; load mid-kernel
                    nc.sync.dma_start(
                        wo_sb[:], wo_d.rearrange("(h p) n -> p h n", p=P)
                    )

                # ---- Phase 2: causal attention, two heads interleaved ----
                for hp in range(HB // 2):
                    pair = (2 * hp, 2 * hp + 1)  # local head idx within block
                    for qc in range(T // 512):
                        jmax = (qc + 1) * 4
                        att = {}
                        den = {}
                        for l in pair:
                            att[l] = ps.tile(
                                [P, 512], F32, tag="att", bufs=3, name=f"att{l}"
                            )
                            den[l] = ps.tile(
                                [P, 512], F32, tag="den", bufs=2, name=f"den{l}"
                            )

                        def bounds(j):
                            s = max(512 * qc, 128 * j)
                            return s, 512 * qc + 512 - s

                        sts = {}

                        def issue_st(l, j):
                            s, n = bounds(j)
                            st = ps.tile([P, 512], F32, tag="mm", name=f"st{l}")
                            nc.tensor.matmul(
                                st[:, :n],
                                kf[:, l, j * P : (j + 1) * P],
                                qf[:, l, s : 512 * qc + 512],
                                start=True,
                                stop=True,
                            )
                            if 128 * j >= 512 * qc:
                                nc.vector.tensor_tensor(
                                    st[:, :P], st[:, :P], mask_sb[:], ALU.add
                                )
                            sts[(l, j)] = st

                        for l in pair:
                            issue_st(l, 0)
                        for j in range(jmax):
                            s, n = bounds(j)
                            c0 = s - 512 * qc
                            for l in pair:
                                st = sts.pop((l, j))
                                E = wp.tile([P, 512], F16, tag="E", bufs=6)
                                nc.scalar.activation(E[:, :n], st[:, :n], AF.Exp)
                                if j + 1 < jmax:
                                    issue_st(l, j + 1)
                                nc.tensor.matmul(
                                    att[l][:, c0:],
                                    vt[:, j, l * DH : (l + 1) * DH],
                                    E[:, :n],
                                    start=(j == 0),
                                    stop=(j == jmax - 1),
                                )
                                nc.tensor.matmul(
                                    den[l][:, c0:],
                                    ones_sb[:],
                                    E[:, :n],
                                    start=(j == 0),
                                    stop=(j == jmax - 1),
                                )
                        for l in pair:
                            hh = blk * HB + l
                            rc = wp.tile([P, 512], F32, tag="rc")
                            nc.vector.reciprocal_approx_fast(rc[:], den[l][:])
                            nc.vector.tensor_tensor(
                                oT[:, hh, qc * 512 : (qc + 1) * 512],
                                att[l][:],
                                rc[:],
                                ALU.mult,
                            )

            # ---- Phase 3: out projection (partial over this core's heads) ----
            part_v = part_d.rearrange("(mo p) n -> p mo n", p=P)
            for m in range(T // P):
                for n2 in range(C // 512):
                    pt = ps.tile([P, 512], F32, tag="mm")
                    for h in range(HL):
                        nc.tensor.matmul(
                            pt[:],
                            oT[:, h, m * P : (m + 1) * P],
                            wo_sb[:, h, n2 * 512 : (n2 + 1) * 512],
                            start=(h == 0),
                            stop=(h == HL - 1),
                        )
                    po = wp.tile([P, 512], F32, tag="po")
                    nc.vector.tensor_copy(po[:], pt[:])
                    nc.sync.dma_start(part_v[:, m, n2 * 512 : (n2 + 1) * 512], po[:])

    nc.compile()
    return nc


def _prep_inputs(x, w_qkv, b_qkv, w_out):
    """Build the 8 per-core input maps (host-side shard + layout prep)."""
    f16 = np.float16
    scale = np.float32(1.0 / np.sqrt(DH))

    xt = [np.ascontiguousarray(x[b].T).astype(f16) for b in range(B)]

    mask = np.where(
        np.arange(P)[None, :] >= np.arange(P)[:, None], 0.0, -1e30
    ).astype(np.float32)

    per_g = []
    for g in range(2):
        lo, hi = g * HL * DH, (g + 1) * HL * DH
        wq = np.ascontiguousarray(w_qkv[:, lo:hi] * scale).astype(f16)
        wk = np.ascontiguousarray(w_qkv[:, C + lo : C + hi]).astype(f16)
        wv = np.ascontiguousarray(w_qkv[:, 2 * C + lo : 2 * C + hi]).astype(f16)
        wo = np.ascontiguousarray(w_out[lo:hi, :]).astype(f16)
        bq = (b_qkv[lo:hi] * scale).astype(np.float32).reshape(HL, P).T.copy()
        bk = b_qkv[C + lo : C + hi].astype(np.float32).reshape(HL, P).T.copy()
        bv = b_qkv[2 * C + lo : 2 * C + hi].astype(np.float32)
        bvb = np.ascontiguousarray(np.broadcast_to(bv[None, :], (P, HL * DH)))
        per_g.append(dict(wq=wq, wk=wk, wv=wv, wo=wo, bq=bq, bk=bk, bvb=bvb))

    in_maps = []
    for c in range(NCORES):
        b, g = c // 2, c % 2
        m = dict(per_g[g])
        m["xt"] = xt[b]
        m["mask"] = mask
        in_maps.append(m)
    return in_maps


def run(x, w_qkv, b_qkv, w_out, b_out, trace=False, **trace_kwargs):
    from concourse.bass_utils import run_bass_kernel_spmd

    x = np.asarray(x, dtype=np.float32)
    w_qkv = np.asarray(w_qkv, dtype=np.float32)
    b_qkv = np.asarray(b_qkv, dtype=np.float32)
    w_out = np.asarray(w_out, dtype=np.float32)
    b_out = np.asarray(b_out, dtype=np.float32)

    if "nc" not in _cache:
        _cache["nc"] = _build()
    nc = _cache["nc"]

    in_maps = _prep_inputs(x, w_qkv, b_qkv, w_out)
    res = run_bass_kernel_spmd(
        nc, in_maps, core_ids=list(range(NCORES)), trace=trace, **trace_kwargs
    )

    out = np.empty((B, T, C), np.float32)
    for b in range(B):
        out[b] = res.results[2 * b]["part"] + res.results[2 * b + 1]["part"]
    out += b_out
    return out, res


def kernel(x, w_qkv, b_qkv, w_out, b_out):
    out, _ = run(x, w_qkv, b_qkv, w_out, b_out)
    return out



# revision 2
# speedup vs baseline: 1.0395x; 1.0395x over previous
"""Multi-head causal self-attention (B=4, T=1024, d_model=2048, 16 heads of 128)
for 8 Trainium2 NeuronCores.

Sharding: hybrid data x tensor parallel. Core c handles batch b = c//2 and
head group g = c%2 (8 heads per core). Each core computes q/k/v projections
for its 8 heads, causal flash-style attention, and the out-projection rows
for those heads, producing a partial [1024, 2048] output for its batch.
The host sums the two partials per batch and adds the output bias.

Performance structure (v2):
  - q/k projections run in fp8(e4m3) with DoubleRow perf mode: each matmul
    contracts 256 rows (2x128 pairs) per pass, halving PE time. Weights are
    pre-scaled by 32 on the host to center fp8's dynamic range; the descale
    (and the bias add) is folded into the PSUM-drain on the Scalar engine
    via activation(Identity, bias, scale). Only q/k can take fp8: the output
    max-error metric rides a 42-sigma outlier that flows through V and the
    out-projection almost verbatim, while q/k errors only perturb softmax
    scores (measured end-to-end rel err 1.1e-2 vs the 2e-2 gate).
  - v projection and out-projection stay fp16 (precision-critical path).
  - The softmax denominator is no longer a per-chunk ones-matmul on the PE:
    exp chunks are accumulated on the Vector engine into E_acc and a single
    ones-matmul per (head, q-chunk) reduces+broadcasts the sum (PE work /4.5).
  - Output partials ship fp16 (host sums in fp32) to halve output DMA.
  - q/k loops run kc-outermost over head pairs so the first block's matmuls
    consume input chunks at ~DMA arrival rate instead of stalling, and so
    consecutive matmuls share LDWEIGHTS (t=0,1 reuse the stationary operand).

All on-device layouts are feature-major so no transposes are needed anywhere:
  - x is shipped pre-transposed per batch: xt8 (fp8) for q/k, xt (fp16) for v
  - q, k are produced feature-major [dh, T] per head; v token-major [T, dh]
  - scores are computed transposed: S^T[kv, q] = k_fm.T @ q_fm (lhsT=k, rhs=q)
  - attention output accumulates as out^T[dh, q] = v_tm.T @ exp(S^T)
  - out^T is exactly the lhsT the out-projection needs
"""

import numpy as np

B, T, C = 4, 1024, 2048
H = 16          # total heads
HL = 8          # heads per core (local)
HB = 4          # heads per block
DH = 128        # head dim
KC = C // 128   # fp16 contraction chunks (16)
KC8 = C // 256  # fp8 DoubleRow pair chunks (8)
P = 128
NCORES = 8
WS = 32.0       # fp8 weight pre-scale (power of two)

_cache = {}


def _build():
    import concourse.bacc as bacc
    import concourse.mybir as mybir
    import concourse.tile as tile

    F32 = mybir.dt.float32
    F16 = mybir.dt.float16
    F8 = mybir.dt.float8e4
    AF = mybir.ActivationFunctionType
    ALU = mybir.AluOpType
    DR = mybir.MatmulPerfMode.DoubleRow

    scale = float(1.0 / np.sqrt(DH))

    nc = bacc.Bacc("TRN2", target_bir_lowering=False, debug=False)

    xt8_d = nc.dram_tensor("xt8", (C, T), F8, kind="ExternalInput")
    xt_d = nc.dram_tensor("xt", (C, T), F16, kind="ExternalInput")
    wq8_d = nc.dram_tensor("wq8", (C, HL * DH), F8, kind="ExternalInput")
    wk8_d = nc.dram_tensor("wk8", (C, HL * DH), F8, kind="ExternalInput")
    wv_d = nc.dram_tensor("wv", (C, HL * DH), F16, kind="ExternalInput")
    wo_d = nc.dram_tensor("wo", (HL * DH, C), F16, kind="ExternalInput")
    bq_d = nc.dram_tensor("bq", (P, HL), F32, kind="ExternalInput")
    bk_d = nc.dram_tensor("bk", (P, HL), F32, kind="ExternalInput")
    bvb_d = nc.dram_tensor("bvb", (P, HL * DH), F32, kind="ExternalInput")
    mask_d = nc.dram_tensor("mask", (P, P), F32, kind="ExternalInput")
    part_d = nc.dram_tensor("part", (T, C), F16, kind="ExternalOutput")

    BW = HB * DH  # head-block feature width (512)

    xt8_v = xt8_d.rearrange("(o p) t -> p o t", p=P)
    xt_v = xt_d.rearrange("(o p) t -> p o t", p=P)
    wq8_v = wq8_d.rearrange("(o p) m -> p o m", p=P)
    wk8_v = wk8_d.rearrange("(o p) m -> p o m", p=P)
    wv_v = wv_d.rearrange("(o p) m -> p o m", p=P)

    with tile.TileContext(nc) as tc:
        with (
            tc.tile_pool(name="res", bufs=1) as res,
            tc.tile_pool(name="wblk", bufs=1) as wblk,
            tc.tile_pool(name="qkv", bufs=2) as qkv,
            tc.tile_pool(name="wp", bufs=3) as wp,
            tc.tile_pool(name="ps", bufs=5, space="PSUM") as ps,
        ):
            bq_sb = res.tile([P, HL], F32, tag="bq")
            bk_sb = res.tile([P, HL], F32, tag="bk")
            bvb_sb = res.tile([P, HL * DH], F32, tag="bvb")
            mask_sb = res.tile([P, P], F32, tag="mask")

            ones_sb = res.tile([P, P], F16, tag="ones")
            nc.vector.memset(ones_sb[:], 1.0)

            # Warm the PE (HAM un-throttles after ~3.4us of activity) while the
            # input DMAs stream in; these matmuls depend only on the memset.
            warm = ps.tile([P, P], F32, tag="mm")
            for _ in range(48):
                nc.tensor.matmul(warm[:], ones_sb[:], ones_sb[:], start=True, stop=True)

            # x^T: fp8 pair-chunks for q/k DoubleRow, fp16 chunks for v
            xt8s = []
            for kc in range(KC8):
                t8 = res.tile([P, 2, T], F8, tag=f"xt8{kc}", name=f"xt8{kc}")
                xt8s.append(t8)
            xts = []
            for kc in range(KC):
                xt_sb = res.tile([P, T], F16, tag=f"xt{kc}", name=f"xt{kc}")
                xts.append(xt_sb)
            w8ts = {w: [None] * KC8 for w in ("wq8", "wk8")}
            wvts = [None] * KC

            def load_w8(wname, wv_, kc, blk):
                lo = blk * BW
                wt = wblk.tile([P, 2, BW], F8, tag=f"{wname}{kc}", name=f"{wname}{kc}_{blk}")
                nc.sync.dma_start(wt[:, 0, :], wv_[:, 2 * kc, lo : lo + BW])
                nc.sync.dma_start(wt[:, 1, :], wv_[:, 2 * kc + 1, lo : lo + BW])
                w8ts[wname][kc] = wt

            def load_wv(kc, blk):
                lo = blk * BW
                wt = wblk.tile([P, BW], F16, tag=f"wv{kc}", name=f"wv{kc}_{blk}")
                nc.sync.dma_start(wt[:], wv_v[:, kc, lo : lo + BW])
                wvts[kc] = wt

            def dma_block_weights(blk):
                if blk == 0:
                    # arrival order matches first consumption: q-pair-0's
                    # kc-outer loop needs complete (xt8[kc], wq8[kc]) chunks
                    for kc in range(KC8):
                        nc.sync.dma_start(xt8s[kc][:, 0, :], xt8_v[:, 2 * kc, :])
                        nc.sync.dma_start(xt8s[kc][:, 1, :], xt8_v[:, 2 * kc + 1, :])
                        load_w8("wq8", wq8_v, kc, blk)
                    nc.sync.dma_start(bq_sb[:], bq_d[:])
                    nc.sync.dma_start(bk_sb[:], bk_d[:])
                    nc.sync.dma_start(bvb_sb[:], bvb_d[:])
                    nc.sync.dma_start(mask_sb[:], mask_d[:])
                    for kc in range(KC8):
                        load_w8("wk8", wk8_v, kc, blk)
                    for kc in range(KC):
                        nc.sync.dma_start(xts[kc][:], xt_v[:, kc, :])
                    for kc in range(KC):
                        load_wv(kc, blk)
                else:
                    for kc in range(KC8):
                        load_w8("wq8", wq8_v, kc, blk)
                        load_w8("wk8", wk8_v, kc, blk)
                    for kc in range(KC):
                        load_wv(kc, blk)

            wo_sb = res.tile([P, HL, C], F16, tag="wo")
            oT = res.tile([P, HL, T], F16, tag="oT")

            for blk in range(HL // HB):
                lo = blk * BW
                dma_block_weights(blk)

                qf = qkv.tile([P, HB, T], F16, tag="qf")
                kf = qkv.tile([P, HB, T], F16, tag="kf")
                vt = qkv.tile([P, T // P, BW], F16, tag="vt")

                # ---- Phase 1a: q/k projections, fp8 DoubleRow, kc-outer ----
                # Head pairs with kc outermost: each (xt8,w8) chunk feeds 4
                # matmuls as it lands, and t=0/1 share the stationary operand.
                for hp in range(HB // 2):
                    for dst, wname, bsb, sc in (
                        ("qf", "wq8", bq_sb, scale / WS),
                        ("kf", "wk8", bk_sb, 1.0 / WS),
                    ):
                        dtile = qf if dst == "qf" else kf
                        pts = []
                        for h2 in range(2):
                            for t in range(2):
                                pt = ps.tile(
                                    [P, 512], F32, tag="mm", name=f"p{dst}{hp}{h2}{t}"
                                )
                                pts.append(pt)
                        for kc in range(KC8):
                            for h2 in range(2):
                                h = 2 * hp + h2
                                for t in range(2):
                                    nc.tensor.matmul(
                                        pts[2 * h2 + t][:],
                                        w8ts[wname][kc][:, :, h * DH : (h + 1) * DH],
                                        xt8s[kc][:, :, t * 512 : (t + 1) * 512],
                                        start=(kc == 0),
                                        stop=(kc == KC8 - 1),
                                        perf_mode=DR,
                                    )
                        for h2 in range(2):
                            h = 2 * hp + h2
                            for t in range(2):
                                nc.scalar.activation(
                                    dtile[:, h, t * 512 : (t + 1) * 512],
                                    pts[2 * h2 + t][:],
                                    AF.Identity,
                                    bias=bsb[:, blk * HB + h : blk * HB + h + 1],
                                    scale=sc,
                                )

                # ---- Phase 1b: v projection, fp16 ----
                for m in range(T // P):
                    pt = ps.tile([P, 512], F32, tag="mm")
                    for kc in range(KC):
                        nc.tensor.matmul(
                            pt[:],
                            xts[kc][:, m * P : (m + 1) * P],
                            wvts[kc][:],
                            start=(kc == 0),
                            stop=(kc == KC - 1),
                        )
                    nc.vector.tensor_tensor(
                        vt[:, m, :], pt[:], bvb_sb[:, lo : lo + BW], ALU.add
                    )

                if blk == 0:
                    # out-proj weights: needed only in phase 3; load mid-kernel
                    nc.sync.dma_start(
                        wo_sb[:], wo_d.rearrange("(h p) n -> p h n", p=P)
                    )

                # ---- Phase 2: causal attention, two heads interleaved ----
                for hp in range(HB // 2):
                    pair = (2 * hp, 2 * hp + 1)  # local head idx within block
                    for qc in range(T // 512):
                        jmax = (qc + 1) * 4
                        att = {}
                        eacc = {}
                        for l in pair:
                            att[l] = ps.tile(
                                [P, 512], F32, tag="att", bufs=3, name=f"att{l}"
                            )
                            eacc[l] = wp.tile(
                                [P, 512], F16, tag="eacc", bufs=3, name=f"eacc{l}"
                            )

                        def bounds(j):
                            s = max(512 * qc, 128 * j)
                            return s, 512 * qc + 512 - s

                        sts = {}

                        def issue_st(l, j):
                            s, n = bounds(j)
                            st = ps.tile([P, 512], F32, tag="mm", name=f"st{l}")
                            nc.tensor.matmul(
                                st[:, :n],
                                kf[:, l, j * P : (j + 1) * P],
                                qf[:, l, s : 512 * qc + 512],
                                start=True,
                                stop=True,
                            )
                            if 128 * j >= 512 * qc:
                                nc.vector.tensor_tensor(
                                    st[:, :P], st[:, :P], mask_sb[:], ALU.add
                                )
                            sts[(l, j)] = st

                        for l in pair:
                            issue_st(l, 0)
                        for j in range(jmax):
                            s, n = bounds(j)
                            c0 = s - 512 * qc
                            for l in pair:
                                st = sts.pop((l, j))
                                E = wp.tile([P, 512], F16, tag="E", bufs=6)
                                nc.scalar.activation(E[:, :n], st[:, :n], AF.Exp)
                                if j + 1 < jmax:
                                    issue_st(l, j + 1)
                                nc.tensor.matmul(
                                    att[l][:, c0:],
                                    vt[:, j, l * DH : (l + 1) * DH],
                                    E[:, :n],
                                    start=(j == 0),
                                    stop=(j == jmax - 1),
                                )
                                if j == 0:
                                    nc.vector.tensor_copy(eacc[l][:], E[:])
                                else:
                                    nc.vector.tensor_tensor(
                                        eacc[l][:, c0:],
                                        eacc[l][:, c0:],
                                        E[:, :n],
                                        ALU.add,
                                    )
                        for l in pair:
                            hh = blk * HB + l
                            den = ps.tile([P, 512], F32, tag="mm", name=f"den{l}")
                            nc.tensor.matmul(
                                den[:], ones_sb[:], eacc[l][:], start=True, stop=True
                            )
                            rc = wp.tile([P, 512], F32, tag="rc")
                            nc.vector.reciprocal_approx_fast(rc[:], den[:])
                            nc.vector.tensor_tensor(
                                oT[:, hh, qc * 512 : (qc + 1) * 512],
                                att[l][:],
                                rc[:],
                                ALU.mult,
                            )

            # ---- Phase 3: out projection (partial over this core's heads) ----
            part_v = part_d.rearrange("(mo p) n -> p mo n", p=P)
            for m in range(T // P):
                for n2 in range(C // 512):
                    pt = ps.tile([P, 512], F32, tag="mm")
                    for h in range(HL):
                        nc.tensor.matmul(
                            pt[:],
                            oT[:, h, m * P : (m + 1) * P],
                            wo_sb[:, h, n2 * 512 : (n2 + 1) * 512],
                            start=(h == 0),
                            stop=(h == HL - 1),
                        )
                    po = wp.tile([P, 512], F16, tag="po")
                    nc.vector.tensor_copy(po[:], pt[:])
                    nc.sync.dma_start(part_v[:, m, n2 * 512 : (n2 + 1) * 512], po[:])

    nc.compile()
    return nc


def _prep_inputs(x, w_qkv, b_qkv, w_out):
    """Build the 8 per-core input maps (host-side shard + layout prep)."""
    import ml_dtypes

    f16 = np.float16
    f8 = ml_dtypes.float8_e4m3
    scale = np.float32(1.0 / np.sqrt(DH))

    xt16 = [np.ascontiguousarray(x[b].T).astype(f16) for b in range(B)]
    xt8 = [np.ascontiguousarray(x[b].T).astype(f8) for b in range(B)]

    mask = np.where(
        np.arange(P)[None, :] >= np.arange(P)[:, None], 0.0, -1e30
    ).astype(np.float32)

    per_g = []
    for g in range(2):
        lo, hi = g * HL * DH, (g + 1) * HL * DH
        wq8 = np.ascontiguousarray(w_qkv[:, lo:hi] * WS).astype(f8)
        wk8 = np.ascontiguousarray(w_qkv[:, C + lo : C + hi] * WS).astype(f8)
        wv = np.ascontiguousarray(w_qkv[:, 2 * C + lo : 2 * C + hi]).astype(f16)
        wo = np.ascontiguousarray(w_out[lo:hi, :]).astype(f16)
        bq = (b_qkv[lo:hi] * scale).astype(np.float32).reshape(HL, P).T.copy()
        bk = b_qkv[C + lo : C + hi].astype(np.float32).reshape(HL, P).T.copy()
        bv = b_qkv[2 * C + lo : 2 * C + hi].astype(np.float32)
        bvb = np.ascontiguousarray(np.broadcast_to(bv[None, :], (P, HL * DH)))
        per_g.append(dict(wq8=wq8, wk8=wk8, wv=wv, wo=wo, bq=bq, bk=bk, bvb=bvb))

    in_maps = []
    for c in range(NCORES):
        b, g = c // 2, c % 2
        m = dict(per_g[g])
        m["xt"] = xt16[b]
        m["xt8"] = xt8[b]
        m["mask"] = mask
        in_maps.append(m)
    return in_maps


def run(x, w_qkv, b_qkv, w_out, b_out, trace=False, **trace_kwargs):
    from concourse.bass_utils import run_bass_kernel_spmd

    x = np.asarray(x, dtype=np.float32)
    w_qkv = np.asarray(w_qkv, dtype=np.float32)
    b_qkv = np.asarray(b_qkv, dtype=np.float32)
    w_out = np.asarray(w_out, dtype=np.float32)
    b_out = np.asarray(b_out, dtype=np.float32)

    if "nc" not in _cache:
        _cache["nc"] = _build()
    nc = _cache["nc"]

    in_maps = _prep_inputs(x, w_qkv, b_qkv, w_out)
    res = run_bass_kernel_spmd(
        nc, in_maps, core_ids=list(range(NCORES)), trace=trace, **trace_kwargs
    )

    out = np.empty((B, T, C), np.float32)
    for b in range(B):
        out[b] = res.results[2 * b]["part"].astype(np.float32) + res.results[
            2 * b + 1
        ]["part"].astype(np.float32)
    out += b_out
    return out, res


def kernel(x, w_qkv, b_qkv, w_out, b_out):
    out, _ = run(x, w_qkv, b_qkv, w_out, b_out)
    return out


# revision 3
# speedup vs baseline: 1.2262x; 1.1796x over previous
"""Multi-head causal self-attention (B=4, T=1024, d_model=2048, 16 heads of 128)
for 8 Trainium2 NeuronCores.

Sharding: hybrid data x tensor parallel. Core c handles batch b = c//2 and
head group g = c%2 (8 heads per core). Each core computes q/k/v projections
for its 8 heads, causal flash-style attention, and the out-projection rows
for those heads, producing a partial [1024, 2048] output for its batch.
The host sums the two partials per batch and adds the output bias.

Performance structure (v3):
  - q/k projections run in fp8(e4m3) with DoubleRow perf mode: each matmul
    contracts 256 rows (2x128 pairs) per pass, halving PE time. Weights are
    pre-scaled by 32 on the host to center fp8's dynamic range; the descale
    and bias add are folded into the PSUM-drain on the Scalar engine via
    activation(Identity, bias, scale). Only q/k can take fp8: the output
    max-error metric rides a 42-sigma outlier that flows through V and the
    out-projection almost verbatim, while q/k errors only perturb softmax
    scores (measured end-to-end rel err 1.1e-2 vs the 2e-2 gate).
  - v projection and out-projection stay fp16 (precision-critical path).
  - Softmax denominator: exp chunks are accumulated on the Vector engine
    into E_acc and a single ones-matmul per (head, q-chunk) reduces and
    broadcasts the sum (PE work /4.5 vs a ones-matmul per kv chunk).
  - DMA issue is the hidden serializer: each dma_start costs ~0.7us on the
    issuing sequencer. All inputs are shipped in large contiguous-per-chunk
    layouts and loaded with ~20 batched dma_starts spread over both HWDGE
    rings (sync + scalar), with next-block weights on gpsimd (SWDGE) so
    their write-after-read waits never block the sync ring.
  - Emission is software-pipelined across phase boundaries: block-1 q/k
    matmuls are emitted before block-0's last softmax-denominator matmuls
    (which wait on the DVE accumulation chain), and the first out-projection
    groups are emitted around block-1's last denominator, so the in-order
    PE queue never runs dry at phase transitions.
  - Output partials ship fp16 (host sums in fp32) to halve output DMA.

All on-device layouts are feature-major so no transposes are needed anywhere:
  - x is shipped pre-transposed per batch: xt8 (fp8 pair-chunks) for q/k,
    xt (fp16) for v
  - q, k are produced feature-major [dh, T] per head; v token-major [T, dh]
  - scores are computed transposed: S^T[kv, q] = k_fm.T @ q_fm (lhsT=k, rhs=q)
  - attention output accumulates as out^T[dh, q] = v_tm.T @ exp(S^T)
  - out^T is exactly the lhsT the out-projection needs
"""

import numpy as np

B, T, C = 4, 1024, 2048
H = 16          # total heads
HL = 8          # heads per core (local)
HB = 4          # heads per block
DH = 128        # head dim
KC = C // 128   # fp16 contraction chunks (16)
KC8 = C // 256  # fp8 DoubleRow pair chunks (8)
P = 128
NCORES = 8
WS = 32.0       # fp8 weight pre-scale (power of two)
BW = HB * DH    # head-block feature width (512)

_cache = {}


def _build():
    import concourse.bacc as bacc
    import concourse.mybir as mybir
    import concourse.tile as tile

    F32 = mybir.dt.float32
    F16 = mybir.dt.float16
    F8 = mybir.dt.float8e4
    AF = mybir.ActivationFunctionType
    ALU = mybir.AluOpType
    DR = mybir.MatmulPerfMode.DoubleRow

    scale = float(1.0 / np.sqrt(DH))

    nc = bacc.Bacc("TRN2", target_bir_lowering=False, debug=False)

    # [kc][p][i][t] = x^T[256*kc + 128*i + p, t], fp8
    xt8_d = nc.dram_tensor("xt8", (KC8 * P, 2 * T), F8, kind="ExternalInput")
    xt_d = nc.dram_tensor("xt", (C, T), F16, kind="ExternalInput")
    # [b][kc][p][i][m] = w[256*kc + 128*i + p, b*512 + m] * WS, fp8
    wq8_d = nc.dram_tensor("wq8", (2 * KC8 * P, 2 * BW), F8, kind="ExternalInput")
    wk8_d = nc.dram_tensor("wk8", (2 * KC8 * P, 2 * BW), F8, kind="ExternalInput")
    wv_d = nc.dram_tensor("wv", (C, HL * DH), F16, kind="ExternalInput")
    wo_d = nc.dram_tensor("wo", (HL * DH, C), F16, kind="ExternalInput")
    # packed per-partition constants: bq[0:8] bk[8:16] bvb[16:1040] mask[1040:1168]
    bias_d = nc.dram_tensor("biases", (P, 2 * HL + HL * DH + P), F32, kind="ExternalInput")
    part_d = nc.dram_tensor("part", (T, C), F16, kind="ExternalOutput")

    xt8_v = xt8_d.rearrange("(k p) (i t) -> p k i t", p=P, i=2)
    xt_v = xt_d.rearrange("(o p) t -> p o t", p=P)
    wq8_v = wq8_d.rearrange("(b k p) (i m) -> p b k i m", b=2, k=KC8, i=2)
    wk8_v = wk8_d.rearrange("(b k p) (i m) -> p b k i m", b=2, k=KC8, i=2)
    wv_v = wv_d.rearrange("(o p) m -> p o m", p=P)

    with tile.TileContext(nc) as tc:
        with (
            tc.tile_pool(name="res", bufs=1) as res,
            tc.tile_pool(name="wblk", bufs=1) as wblk,
            tc.tile_pool(name="qkv", bufs=2) as qkv,
            tc.tile_pool(name="wp", bufs=3) as wp,
            tc.tile_pool(name="ps", bufs=5, space="PSUM") as ps,
        ):
            bias_sb = res.tile([P, 2 * HL + HL * DH + P], F32, tag="biases")
            BQ, BK, BVB, MSK = 0, HL, 2 * HL, 2 * HL + HL * DH

            ones_sb = res.tile([P, P], F16, tag="ones")
            nc.vector.memset(ones_sb[:], 1.0)

            # Warm the PE (HAM un-throttles after ~3.4us of activity) while the
            # input DMAs stream in; these matmuls depend only on the memset.
            warm = ps.tile([P, P], F32, tag="mm")
            for _ in range(36):
                nc.tensor.matmul(warm[:], ones_sb[:], ones_sb[:], start=True, stop=True)

            xt8_sb = res.tile([P, KC8, 2, T], F8, tag="xt8")
            xt16_sb = res.tile([P, KC, T], F16, tag="xt16")
            wo_sb = res.tile([P, HL, C], F16, tag="wo")
            oT = res.tile([P, HL, T], F16, tag="oT")

            w8ts = {}
            wv_ts = {}

            def dma_in_blk0():
                # sync HWDGE ring: the fp8 q/k stream, interleaved so the
                # kc-outer q loop can start after the first two chunk-pairs
                nc.sync.dma_start(xt8_sb[:, 0:2, :, :], xt8_v[:, 0:2, :, :])
                nc.sync.dma_start(wq8_sb_0[:, 0:4, :, :], wq8_v[:, 0, 0:4, :, :])
                nc.sync.dma_start(xt8_sb[:, 2:4, :, :], xt8_v[:, 2:4, :, :])
                nc.sync.dma_start(bias_sb[:], bias_d[:])
                nc.sync.dma_start(xt8_sb[:, 4:6, :, :], xt8_v[:, 4:6, :, :])
                nc.sync.dma_start(wq8_sb_0[:, 4:8, :, :], wq8_v[:, 0, 4:8, :, :])
                nc.sync.dma_start(xt8_sb[:, 6:8, :, :], xt8_v[:, 6:8, :, :])
                nc.sync.dma_start(wk8_sb_0[:, 0:4, :, :], wk8_v[:, 0, 0:4, :, :])
                nc.sync.dma_start(wk8_sb_0[:, 4:8, :, :], wk8_v[:, 0, 4:8, :, :])
                # scalar HWDGE ring: the fp16 v / out-proj stream in parallel
                nc.scalar.dma_start(xt16_sb[:, 0:8, :], xt_v[:, 0:8, :])
                nc.scalar.dma_start(xt16_sb[:, 8:16, :], xt_v[:, 8:16, :])
                nc.scalar.dma_start(wv_sb_0[:], wv_v[:, :, 0:BW])
                nc.scalar.dma_start(wo_sb[:], wo_d.rearrange("(h p) n -> p h n", p=P))

            def dma_in_blk1():
                # gpsimd SWDGE: waits (write-after-read on the blk0 tiles)
                # park on the idle GpSimd queue instead of blocking sync
                nc.gpsimd.dma_start(wq8_sb_1[:], wq8_v[:, 1, :, :, :])
                nc.gpsimd.dma_start(wk8_sb_1[:], wk8_v[:, 1, :, :, :])
                nc.gpsimd.dma_start(wv_sb_1[:], wv_v[:, :, BW : 2 * BW])

            wq8_sb_0 = wblk.tile([P, KC8, 2, BW], F8, tag="wq8", name="wq8_0")
            wk8_sb_0 = wblk.tile([P, KC8, 2, BW], F8, tag="wk8", name="wk8_0")
            wv_sb_0 = wblk.tile([P, KC, BW], F16, tag="wv", name="wv_0")
            wq8_sb_1 = wblk.tile([P, KC8, 2, BW], F8, tag="wq8", name="wq8_1")
            wk8_sb_1 = wblk.tile([P, KC8, 2, BW], F8, tag="wk8", name="wk8_1")
            wv_sb_1 = wblk.tile([P, KC, BW], F16, tag="wv", name="wv_1")
            w8ts = {(0, "q"): wq8_sb_0, (0, "k"): wk8_sb_0,
                    (1, "q"): wq8_sb_1, (1, "k"): wk8_sb_1}
            wv_ts = {0: wv_sb_0, 1: wv_sb_1}

            dma_in_blk0()
            dma_in_blk1()

            qfs, kfs, vts = {}, {}, {}

            def proj_qk_pair(blk, hp):
                """q then k projections for head pair hp of block blk.
                fp8 DoubleRow, kc outermost: each chunk feeds 4 matmuls as it
                lands and t=0/1 share the stationary operand."""
                if hp == 0:
                    qfs[blk] = qkv.tile([P, HB, T], F16, tag="qf", name=f"qf{blk}")
                    kfs[blk] = qkv.tile([P, HB, T], F16, tag="kf", name=f"kf{blk}")
                for dst, wkey, boff, sc in (
                    ("q", "q", BQ, scale / WS),
                    ("k", "k", BK, 1.0 / WS),
                ):
                    dtile = qfs[blk] if dst == "q" else kfs[blk]
                    wt = w8ts[(blk, wkey)]
                    pts = []
                    for h2 in range(2):
                        for t in range(2):
                            pt = ps.tile(
                                [P, 512], F32, tag="mm", name=f"p{dst}{blk}{hp}{h2}{t}"
                            )
                            pts.append(pt)
                    for kc in range(KC8):
                        for h2 in range(2):
                            h = 2 * hp + h2
                            for t in range(2):
                                nc.tensor.matmul(
                                    pts[2 * h2 + t][:],
                                    wt[:, kc, :, h * DH : (h + 1) * DH],
                                    xt8_sb[:, kc, :, t * 512 : (t + 1) * 512],
                                    start=(kc == 0),
                                    stop=(kc == KC8 - 1),
                                    perf_mode=DR,
                                )
                    for h2 in range(2):
                        h = 2 * hp + h2
                        gh = blk * HB + h
                        for t in range(2):
                            nc.scalar.activation(
                                dtile[:, h, t * 512 : (t + 1) * 512],
                                pts[2 * h2 + t][:],
                                AF.Identity,
                                bias=bias_sb[:, boff + gh : boff + gh + 1],
                                scale=sc,
                            )

            def proj_v(blk):
                vts[blk] = qkv.tile([P, T // P, BW], F16, tag="vt", name=f"vt{blk}")
                vt = vts[blk]
                for m in range(T // P):
                    pt = ps.tile([P, 512], F32, tag="mm")
                    for kc in range(KC):
                        nc.tensor.matmul(
                            pt[:],
                            xt16_sb[:, kc, m * P : (m + 1) * P],
                            wv_ts[blk][:, kc, :],
                            start=(kc == 0),
                            stop=(kc == KC - 1),
                        )
                    nc.vector.tensor_tensor(
                        vt[:, m, :],
                        pt[:],
                        bias_sb[:, BVB + blk * BW : BVB + (blk + 1) * BW],
                        ALU.add,
                    )

            def attn_scores(blk, hp, qc):
                """S^T, exp, E_acc and attention-output accumulation for the
                head pair; returns context for attn_tail."""
                qf, kf, vt = qfs[blk], kfs[blk], vts[blk]
                pair = (2 * hp, 2 * hp + 1)
                jmax = (qc + 1) * 4
                att, eacc = {}, {}
                for l in pair:
                    att[l] = ps.tile([P, 512], F32, tag="att", bufs=3, name=f"att{l}")
                    eacc[l] = wp.tile([P, 512], F16, tag="eacc", bufs=3, name=f"eacc{l}")

                def bounds(j):
                    s = max(512 * qc, 128 * j)
                    return s, 512 * qc + 512 - s

                sts = {}

                def issue_st(l, j):
                    s, n = bounds(j)
                    st = ps.tile([P, 512], F32, tag="mm", name=f"st{l}")
                    nc.tensor.matmul(
                        st[:, :n],
                        kf[:, l, j * P : (j + 1) * P],
                        qf[:, l, s : 512 * qc + 512],
                        start=True,
                        stop=True,
                    )
                    if 128 * j >= 512 * qc:
                        nc.vector.tensor_tensor(
                            st[:, :P], st[:, :P], bias_sb[:, MSK : MSK + P], ALU.add
                        )
                    sts[(l, j)] = st

                for l in pair:
                    issue_st(l, 0)
                for j in range(jmax):
                    s, n = bounds(j)
                    c0 = s - 512 * qc
                    for l in pair:
                        st = sts.pop((l, j))
                        E = wp.tile([P, 512], F16, tag="E", bufs=6)
                        nc.scalar.activation(E[:, :n], st[:, :n], AF.Exp)
                        if j + 1 < jmax:
                            issue_st(l, j + 1)
                        nc.tensor.matmul(
                            att[l][:, c0:],
                            vt[:, j, l * DH : (l + 1) * DH],
                            E[:, :n],
                            start=(j == 0),
                            stop=(j == jmax - 1),
                        )
                        if j == 0:
                            nc.vector.tensor_copy(eacc[l][:], E[:])
                        else:
                            nc.vector.tensor_tensor(
                                eacc[l][:, c0:], eacc[l][:, c0:], E[:, :n], ALU.add
                            )
                return (blk, hp, qc, pair, att, eacc)

            def attn_tail(ctx):
                """Denominator matmul, reciprocal, and oT multiply."""
                blk, hp, qc, pair, att, eacc = ctx
                for l in pair:
                    hh = blk * HB + l
                    den = ps.tile([P, 512], F32, tag="mm", name=f"den{l}")
                    nc.tensor.matmul(
                        den[:], ones_sb[:], eacc[l][:], start=True, stop=True
                    )
                    rc = wp.tile([P, 512], F32, tag="rc")
                    nc.vector.reciprocal_approx_fast(rc[:], den[:])
                    nc.vector.tensor_tensor(
                        oT[:, hh, qc * 512 : (qc + 1) * 512],
                        att[l][:],
                        rc[:],
                        ALU.mult,
                    )

            part_v = part_d.rearrange("(mo p) n -> p mo n", p=P)

            def phase3_group(m, n2, h_list, pt=None, drain=False):
                """Emit out-projection matmuls for chunk (m, n2) over h_list;
                the PSUM group stays open until drain=True finishes it."""
                if pt is None:
                    pt = ps.tile([P, 512], F32, tag="mm", name=f"po{m}{n2}")
                for h in h_list:
                    nc.tensor.matmul(
                        pt[:],
                        oT[:, h, m * P : (m + 1) * P],
                        wo_sb[:, h, n2 * 512 : (n2 + 1) * 512],
                        start=(h == 0),
                        stop=(h == HL - 1),
                    )
                return pt

            pos = {}

            def phase3_drain(m, n2, pt):
                if n2 == 0:
                    pos[m] = wp.tile([P, C], F16, tag="po", bufs=2, name=f"pov{m}")
                po = pos[m]
                nc.vector.tensor_copy(po[:, n2 * 512 : (n2 + 1) * 512], pt[:])
                if n2 == C // 512 - 1:
                    nc.sync.dma_start(part_v[:, m, :], po[:])

            # ---------------- emission schedule ----------------
            proj_qk_pair(0, 0)
            proj_qk_pair(0, 1)
            proj_v(0)

            attn_tail(attn_scores(0, 0, 0))
            attn_tail(attn_scores(0, 0, 1))
            attn_tail(attn_scores(0, 1, 0))
            ctx = attn_scores(0, 1, 1)
            # blk1 q/k fills the PE while blk0's last denominator chain drains
            proj_qk_pair(1, 0)
            attn_tail(ctx)
            proj_qk_pair(1, 1)
            proj_v(1)

            attn_tail(attn_scores(1, 0, 0))
            attn_tail(attn_scores(1, 0, 1))
            attn_tail(attn_scores(1, 1, 0))
            ctx = attn_scores(1, 1, 1)
            # first out-proj group (heads 0..5 ready) fills the last tail;
            # heads 6/7 complete after the deferred denominator
            pt00 = phase3_group(0, 0, range(6))
            pt01 = phase3_group(0, 1, range(6))
            attn_tail(ctx)
            pt00 = phase3_group(0, 0, (6, 7), pt=pt00)
            phase3_drain(0, 0, pt00)
            pt01 = phase3_group(0, 1, (6, 7), pt=pt01)
            phase3_drain(0, 1, pt01)
            for m in range(T // P):
                for n2 in range(C // 512):
                    if m == 0 and n2 < 2:
                        continue
                    pt = phase3_group(m, n2, range(HL))
                    phase3_drain(m, n2, pt)

    nc.compile()
    return nc


def _prep_inputs(x, w_qkv, b_qkv, w_out):
    """Build the 8 per-core input maps (host-side shard + layout prep)."""
    import ml_dtypes

    f16 = np.float16
    f8 = ml_dtypes.float8_e4m3
    scale = np.float32(1.0 / np.sqrt(DH))

    xt16 = [np.ascontiguousarray(x[b].T).astype(f16) for b in range(B)]
    # [kc][p][i][t] = x^T[256kc+128i+p, t]
    xt8 = [
        np.ascontiguousarray(
            x[b].T.reshape(KC8, 2, P, T).transpose(0, 2, 1, 3)
        ).astype(f8).reshape(KC8 * P, 2 * T)
        for b in range(B)
    ]

    mask = np.where(
        np.arange(P)[None, :] >= np.arange(P)[:, None], 0.0, -1e30
    ).astype(np.float32)

    def w8_layout(w):
        # (2048, 1024) -> [b][kc][p][i][m]
        a = (w * WS).reshape(KC8, 2, P, HL * DH).transpose(0, 2, 1, 3)  # k,p,i,m
        a = np.stack([a[..., 0:BW], a[..., BW : 2 * BW]], axis=0)  # b,k,p,i,m
        return np.ascontiguousarray(a).astype(f8).reshape(2 * KC8 * P, 2 * BW)

    per_g = []
    for g in range(2):
        lo, hi = g * HL * DH, (g + 1) * HL * DH
        wq8 = w8_layout(w_qkv[:, lo:hi])
        wk8 = w8_layout(w_qkv[:, C + lo : C + hi])
        wv = np.ascontiguousarray(w_qkv[:, 2 * C + lo : 2 * C + hi]).astype(f16)
        wo = np.ascontiguousarray(w_out[lo:hi, :]).astype(f16)
        bq = (b_qkv[lo:hi] * scale).astype(np.float32).reshape(HL, P).T
        bk = b_qkv[C + lo : C + hi].astype(np.float32).reshape(HL, P).T
        bv = b_qkv[2 * C + lo : 2 * C + hi].astype(np.float32)
        bvb = np.broadcast_to(bv[None, :], (P, HL * DH))
        biases = np.ascontiguousarray(
            np.concatenate([bq, bk, bvb, mask], axis=1)
        ).astype(np.float32)
        per_g.append(dict(wq8=wq8, wk8=wk8, wv=wv, wo=wo, biases=biases))

    in_maps = []
    for c in range(NCORES):
        b, g = c // 2, c % 2
        m = dict(per_g[g])
        m["xt"] = xt16[b]
        m["xt8"] = xt8[b]
        in_maps.append(m)
    return in_maps


def run(x, w_qkv, b_qkv, w_out, b_out, trace=False, **trace_kwargs):
    from concourse.bass_utils import run_bass_kernel_spmd

    x = np.asarray(x, dtype=np.float32)
    w_qkv = np.asarray(w_qkv, dtype=np.float32)
    b_qkv = np.asarray(b_qkv, dtype=np.float32)
    w_out = np.asarray(w_out, dtype=np.float32)
    b_out = np.asarray(b_out, dtype=np.float32)

    if "nc" not in _cache:
        _cache["nc"] = _build()
    nc = _cache["nc"]

    in_maps = _prep_inputs(x, w_qkv, b_qkv, w_out)
    res = run_bass_kernel_spmd(
        nc, in_maps, core_ids=list(range(NCORES)), trace=trace, **trace_kwargs
    )

    out = np.empty((B, T, C), np.float32)
    for b in range(B):
        out[b] = res.results[2 * b]["part"].astype(np.float32) + res.results[
            2 * b + 1
        ]["part"].astype(np.float32)
    out += b_out
    return out, res


def kernel(x, w_qkv, b_qkv, w_out, b_out):
    out, _ = run(x, w_qkv, b_qkv, w_out, b_out)
    return out
